# revision 22
# baseline (speedup 1.0000x reference)
"""Log2Quantizer Trainium2 kernel (raw Bass, no Tile).

Math: the reference's sort/std/rank machinery is dead code (bit_token is
unconditionally overwritten with n_bits), so the computation reduces to:
    delta[b,t] = max over (h,c) of x[b,h,t,c]
    out = delta * 2^(round(log2(max(x/delta, 1e-8))))
i.e. snap x/delta to the nearest power of two in log space, rescale by delta.

Bit-trick (no transcendentals): with q = x * (sqrt2/delta),
    2^round(log2(x/delta)) = 2^floor(log2 q) = bitcast_f32(bits(q) & 0x7F800000)
so   out = delta * (bits(q) & EXP_MASK).  x==0 gives q=0 -> out=0 (the
reference yields delta*2^-27 ~ 7e-9 there; abs err 7e-9).

Engine split + schedule (trace-driven over 7 HW iterations):
  Sync (SP HWDGE ring): loads only.
  DVE: per-token max (tensor_reduce, 1x -- no faster engine or op exists:
       TT-max trees cost the same cycles, tensor_tensor_reduce is
       ISA-length-blocked for strided APs, GpSimd rejects TensorScalarPtr);
       reciprocal (ACT's is banned for accuracy); M1 = (x*inv)*sqrt2
       two-op tensor_scalar for all but the last token-slice (2x_2P);
       AND = exponent mask (bitwise is DVE-only: the BIR verifier rejects
       arith+bitwise fusion). Ops are SOFTWARE-PIPELINED: the next chunk's
       reduce/recip are interleaved between this chunk's M1 slices so every
       RAW fence is already posted when reached.
  ACT: M1B = the last M1 slice of each chunk (activation Copy with
       scale=inv2[P,1]; identity path is 1-ULP fp32 so binning is safe),
       M2 = Copy with scale=delta[P,1] + bf16 output cast, store issuance.
       M2 stays SAME-chunk: a variant pipelining M2 one chunk behind
       serialized the kernel through its WAR waits (11.75us period).
Output is stored as bf16 (harness gate is rel_err < 2e-2; bf16 rounding adds
~1e-3) -> store HBM traffic halves: 25.2MB -> 18.9MB per core.

Chunk schedule [128, 384, 512*6, 256, 256]: tiny first chunk cuts pipeline
fill (the fill phase is load-bandwidth-limited), two small tail chunks
pipeline the drain.

Sharding: data-parallel over batch dim b (8 rows -> 8 cores), no comms.
Layout: partition dim = t-block of tt tokens so each partition line is one
contiguous run per h in DRAM (1KB loads / 512B stores at tt=4).

Sems (every dependent DVE op carries a wait_ge on its producer's inc --
prior session verified HW corruption without the fences; the interleave
just guarantees the waits are already satisfied):
  dve_sem:    +1 per DVE op; absolute per-op indices tracked at trace time
  act_m1_sem: +1 per ACT M1B slice (DVE's AND waits on it)
  act_sem:    +1 per ACT M2 slice; ACT self-fences on it before each store
  load_sem/store_sem[NBUF]: per-slot DMA completion (16/DMA)
"""

from contextlib import ExitStack

import numpy as np

import concourse.bass as bass
import concourse.mybir as mybir
from concourse.bass_utils import run_bass_kernel_spmd

B, H, T, C = 8, 12, 4096, 64
N_CORES = 8
P = 128          # SBUF partitions
NBUF = 6         # xt/wt buffer depth
ROLL = 3         # qt/qt2 rolling-buffer depth (M1/AND -> ACT M2 chain)

_TCS = [128, 384] + [512] * 6 + [256, 256]
CHUNKS = []
_t0 = 0
for _tc in _TCS:
    CHUNKS.append((_t0, _tc))
    _t0 += _tc
assert _t0 == T
TC_MAX = max(_TCS)

SQRT2 = 1.4142135623730951
EXP_MASK = 0x7F800000

_nc_cache = {}


def _build_nc():
    if "nc" in _nc_cache:
        return _nc_cache["nc"]
    f32 = mybir.dt.float32
    bf16 = mybir.dt.bfloat16
    i32 = mybir.dt.int32
    OP = mybir.AluOpType
    AF = mybir.ActivationFunctionType

    nc = bass.Bass()
    x_in = nc.declare_dram_parameter("x", [H, T, C], f32, isOutput=False)
    y_out = nc.declare_dram_parameter("y", [H, T, C], bf16, isOutput=True)

    n = len(CHUNKS)
    TT_MAX = TC_MAX // P
    FREE_MAX = H * TT_MAX * C

    def tt_of(ci):
        return CHUNKS[ci][1] // P

    def m1b_of(ci):
        # ACT runs the last M1 slice when the chunk has more than one
        return 1 if tt_of(ci) > 1 else 0

    # --- absolute dve_sem index per op, computed by simulating the
    # emission order of the vector block below -------------------------
    idx_reduce = [0] * n
    idx_recip = [0] * n     # recip (+ the inv2b scalar when m1b) done
    idx_m1_last = [0] * n
    idx_and = [0] * n
    _c = 0

    def _nxt():
        nonlocal _c
        _c += 1
        return _c

    def _sim_recip(ci):
        idx_recip[ci] = _nxt()          # reciprocal
        if m1b_of(ci):
            idx_recip[ci] = _nxt()      # inv2b = inv * sqrt2 ([P,1])

    idx_reduce[0] = _nxt()
    _sim_recip(0)
    for ci in range(n):
        # iter ci: reduce(ci+1), M1(ci) s0, recip+inv2b(ci+1), M1 rest, AND
        if ci + 1 < n:
            idx_reduce[ci + 1] = _nxt()
        for s in range(tt_of(ci) - m1b_of(ci)):
            idx_m1_last[ci] = _nxt()
            if s == 0 and ci + 1 < n:
                _sim_recip(ci + 1)
        idx_and[ci] = _nxt()

    ACT_END, ACTM1_END = [], []
    _a = _b = 0
    for ci in range(n):
        _a += tt_of(ci)
        _b += m1b_of(ci)
        ACT_END.append(_a)
        ACTM1_END.append(_b)

    def src_ap(ci):
        t0, tc = CHUNKS[ci]
        return x_in[:, t0 : t0 + tc, :].rearrange("h (p q) c -> p h (q c)", p=P)

    def dst_ap(ci):
        t0, tc = CHUNKS[ci]
        return y_out[:, t0 : t0 + tc, :].rearrange("h (p q) c -> p h (q c)", p=P)

    with ExitStack() as ctx:
        xt = [
            ctx.enter_context(nc.sbuf_tensor(f"xt{j}", [P, FREE_MAX], f32))
            for j in range(NBUF)
        ]
        wt = [
            ctx.enter_context(nc.sbuf_tensor(f"wt{j}", [P, FREE_MAX], bf16))
            for j in range(NBUF)
        ]
        qt = [
            ctx.enter_context(nc.sbuf_tensor(f"qt{k}", [P, FREE_MAX], f32))
            for k in range(ROLL)
        ]
        qt2 = [
            ctx.enter_context(nc.sbuf_tensor(f"qt2_{k}", [P, FREE_MAX], f32))
            for k in range(ROLL)
        ]
        delta = [
            ctx.enter_context(nc.sbuf_tensor(f"delta{j}", [P, TT_MAX], f32))
            for j in range(NBUF)
        ]
        # inv[:, :tt] = 1/delta; inv[:, TT_MAX] = inv2b = sqrt2/delta for
        # the ACT M1B slice (ACT's activation has only one scale slot)
        inv = [
            ctx.enter_context(nc.sbuf_tensor(f"inv{j}", [P, TT_MAX + 1], f32))
            for j in range(NBUF)
        ]
        warm = ctx.enter_context(nc.sbuf_tensor("warm", [P, 1], f32))

        load_sem = [
            ctx.enter_context(nc.semaphore(f"load_sem{j}")) for j in range(NBUF)
        ]
        store_sem = [
            ctx.enter_context(nc.semaphore(f"store_sem{j}")) for j in range(NBUF)
        ]
        dve_sem = ctx.enter_context(nc.semaphore("dve_sem"))
        act_m1_sem = ctx.enter_context(nc.semaphore("act_m1_sem"))
        act_sem = ctx.enter_context(nc.semaphore("act_sem"))

        block = ctx.enter_context(nc.Block())

        def views(buf, ci):
            return buf[:, : H * tt_of(ci) * C].rearrange(
                "p (h q c) -> p h q c", h=H, c=C
            )

        @block.sync
        def _(sync):
            for ci in range(n):
                j = ci % NBUF
                if ci >= NBUF:
                    # xt slot readers: reduce + M1 on DVE, M1B on ACT
                    sync.wait_ge(dve_sem, idx_and[ci - NBUF])
                    sync.wait_ge(act_m1_sem, ACTM1_END[ci - NBUF])
                sync.dma_start(
                    out=xt[j][:, : H * tt_of(ci) * C], in_=src_ap(ci)
                ).then_inc(load_sem[j], 16)

        def emit_reduce(vector, ci):
            j = ci % NBUF
            tt = tt_of(ci)
            vector.wait_ge(load_sem[j], 16 * (ci // NBUF + 1))
            if ci >= NBUF:
                # delta slot WAR: M2(ci-NBUF) read it (as scale)
                vector.wait_ge(act_sem, ACT_END[ci - NBUF])
            # delta = max over (h, c): one XY reduce on the [p, q, h, c]
            # transposed view
            vector.reduce_max(
                out=delta[j][:, :tt],
                in_=views(xt[j], ci).transpose([0, 2, 1, 3]),
                axis=mybir.AxisListType.XY,
            ).then_inc(dve_sem, 1)

        def emit_recip(vector, ci):
            j = ci % NBUF
            tt = tt_of(ci)
            vector.wait_ge(dve_sem, idx_reduce[ci])
            vector.reciprocal(inv[j][:, :tt], delta[j][:, :tt]).then_inc(
                dve_sem, 1
            )
            if m1b_of(ci):
                # per-token scalar for ACT's M1B slice ([P,1], ~65 cycles)
                s = tt - 1
                vector.wait_ge(dve_sem, idx_recip[ci] - 1)
                vector.tensor_scalar_mul(
                    inv[j][:, TT_MAX : TT_MAX + 1],
                    inv[j][:, s : s + 1],
                    SQRT2,
                ).then_inc(dve_sem, 1)

        @block.vector
        def _(vector):
            emit_reduce(vector, 0)
            emit_recip(vector, 0)
            for ci in range(n):
                j = ci % NBUF
                tt = tt_of(ci)
                xt4 = views(xt[j], ci)
                qt4 = views(qt[ci % ROLL], ci)

                if ci + 1 < n:
                    emit_reduce(vector, ci + 1)
                if ci >= ROLL:
                    # rolling qt/qt2 + delta WAR: M2(ci-ROLL) must have
                    # read them
                    vector.wait_ge(act_sem, ACT_END[ci - ROLL])
                vector.wait_ge(dve_sem, idx_recip[ci])
                for s in range(tt - m1b_of(ci)):
                    # M1: q = (x * inv) * sqrt2 (two-op tensor_scalar, 2x_2P)
                    vector.tensor_scalar(
                        out=qt4[:, :, s, :],
                        in0=xt4[:, :, s, :],
                        scalar1=inv[j][:, s : s + 1],
                        scalar2=SQRT2,
                        op0=OP.mult,
                        op1=OP.mult,
                    ).then_inc(dve_sem, 1)
                    if s == 0 and ci + 1 < n:
                        emit_recip(vector, ci + 1)
                # AND: p2 = bits(q) & EXP_MASK over the whole chunk (2x_2P);
                # the last M1 slice comes from ACT
                vector.wait_ge(dve_sem, idx_m1_last[ci])
                if m1b_of(ci):
                    vector.wait_ge(act_m1_sem, ACTM1_END[ci])
                vector.tensor_scalar(
                    out=qt2[ci % ROLL][:, : H * tt * C].bitcast(i32),
                    in0=qt[ci % ROLL][:, : H * tt * C].bitcast(i32),
                    scalar1=EXP_MASK,
                    scalar2=None,
                    op0=OP.bitwise_and,
                ).then_inc(dve_sem, 1)

        @block.scalar
        def _(scalar):
            # warm the ACT function table before the pipeline needs it
            scalar.activation(warm[:], warm[:], AF.Copy, scale=1.0)
            for ci in range(n):
                j = ci % NBUF
                tt = tt_of(ci)
                xt4 = views(xt[j], ci)
                qt4 = views(qt[ci % ROLL], ci)
                qt24 = views(qt2[ci % ROLL], ci)
                wt4 = views(wt[j], ci)

                if m1b_of(ci):
                    # M1B: last token-slice of q = x * (sqrt2/delta)
                    s = tt - 1
                    scalar.wait_ge(dve_sem, idx_recip[ci])
                    scalar.activation(
                        out=qt4[:, :, s, :],
                        in_=xt4[:, :, s, :],
                        func=AF.Copy,
                        scale=inv[j][:, TT_MAX : TT_MAX + 1],
                    ).then_inc(act_m1_sem, 1)
                # M2: out = p2 * delta with bf16 cast, then store
                scalar.wait_ge(dve_sem, idx_and[ci])
                if ci >= NBUF:
                    scalar.wait_ge(store_sem[j], 16 * (ci // NBUF))
                for s in range(tt):
                    scalar.activation(
                        out=wt4[:, :, s, :],
                        in_=qt24[:, :, s, :],
                        func=AF.Copy,
                        scale=delta[j][:, s : s + 1],
                    ).then_inc(act_sem, 1)
                # self-fence: M2 writes must land in SBUF before the DMA
                scalar.wait_ge(act_sem, ACT_END[ci])
                scalar.dma_start(
                    out=dst_ap(ci), in_=wt[j][:, : H * tt * C]
                ).then_inc(store_sem[j], 16)

    _nc_cache["nc"] = nc
    return nc


def kernel(x: np.ndarray) -> np.ndarray:
    assert x.shape == (B, H, T, C) and x.dtype == np.float32
    nc = _build_nc()
    in_maps = [{"x": np.ascontiguousarray(x[i])} for i in range(N_CORES)]
    res = run_bass_kernel_spmd(nc, in_maps, list(range(N_CORES)))
    out = np.stack(
        [np.asarray(res.results[i]["y"]).astype(np.float32) for i in range(N_CORES)],
        axis=0,
    )
    return out


# revision 23
# speedup vs baseline: 1.1003x; 1.1003x over previous
"""Log2Quantizer Trainium2 kernel (raw Bass, no Tile).

Math: the reference's sort/std/rank machinery is dead code (bit_token is
unconditionally overwritten with n_bits), so the computation reduces to:
    delta[b,t] = max over (h,c) of x[b,h,t,c]
    out = delta * 2^(round(log2(max(x/delta, 1e-8))))
i.e. snap x/delta to the nearest power of two in log space, rescale by delta.

Bit-trick (no transcendentals): with q = x * (sqrt2/delta),
    2^round(log2(x/delta)) = 2^floor(log2 q) = bitcast_f32(bits(q) & 0x7F800000)
so   out = delta * (bits(q) & EXP_MASK).  x==0 gives q=0 -> out=0 (the
reference yields delta*2^-27 ~ 7e-9 there; abs err 7e-9).

Engine split + schedule (trace-driven over 7 HW iterations):
  Sync (SP HWDGE ring): loads only.
  DVE: per-token max (tensor_reduce, 1x -- no faster engine or op exists:
       TT-max trees cost the same cycles, tensor_tensor_reduce is
       ISA-length-blocked for strided APs, GpSimd rejects TensorScalarPtr);
       reciprocal (ACT's is banned for accuracy); M1 = (x*inv)*sqrt2
       two-op tensor_scalar for all but the last token-slice (2x_2P);
       AND = exponent mask (bitwise is DVE-only: the BIR verifier rejects
       arith+bitwise fusion). Ops are SOFTWARE-PIPELINED: the next chunk's
       reduce/recip are interleaved between this chunk's M1 slices so every
       RAW fence is already posted when reached.
  ACT: M1B = the last M1 slice of each chunk (activation Copy with
       scale=inv2[P,1]; identity path is 1-ULP fp32 so binning is safe),
       M2 = Copy with scale=delta[P,1] + bf16 output cast, store issuance.
       M2 stays SAME-chunk: a variant pipelining M2 one chunk behind
       serialized the kernel through its WAR waits (11.75us period).
Output is stored as bf16 (harness gate is rel_err < 2e-2; bf16 rounding adds
~1e-3) -> store HBM traffic halves: 25.2MB -> 18.9MB per core.

Chunk schedule [128, 384, 512*6, 256, 256]: tiny first chunk cuts pipeline
fill (the fill phase is load-bandwidth-limited), two small tail chunks
pipeline the drain.

Sharding: data-parallel over batch dim b (8 rows -> 8 cores), no comms.
Layout: partition dim = t-block of tt tokens so each partition line is one
contiguous run per h in DRAM (1KB loads / 512B stores at tt=4).

Sems (every dependent DVE op carries a wait_ge on its producer's inc --
prior session verified HW corruption without the fences; the interleave
just guarantees the waits are already satisfied):
  dve_sem:    +1 per DVE op; absolute per-op indices tracked at trace time
  act_m1_sem: +1 per ACT M1B slice (DVE's AND waits on it)
  act_sem:    +1 per ACT M2 slice; ACT self-fences on it before each store
  load_sem/store_sem[NBUF]: per-slot DMA completion (16/DMA)
"""

from contextlib import ExitStack

import numpy as np

import concourse.bass as bass
import concourse.mybir as mybir
from concourse.bass_utils import run_bass_kernel_spmd

B, H, T, C = 8, 12, 4096, 64
N_CORES = 8
P = 128          # SBUF partitions
NBUF = 6         # xt/wt buffer depth
ROLL = 3         # qt/qt2 rolling-buffer depth (M1/AND -> ACT M2 chain)

_TCS = [128, 384] + [512] * 6 + [256, 256]
CHUNKS = []
_t0 = 0
for _tc in _TCS:
    CHUNKS.append((_t0, _tc))
    _t0 += _tc
assert _t0 == T
TC_MAX = max(_TCS)

SQRT2 = 1.4142135623730951
EXP_MASK = 0x7F800000

_nc_cache = {}


def _build_nc():
    if "nc" in _nc_cache:
        return _nc_cache["nc"]
    f32 = mybir.dt.float32
    bf16 = mybir.dt.bfloat16
    i32 = mybir.dt.int32
    OP = mybir.AluOpType
    AF = mybir.ActivationFunctionType

    nc = bass.Bass()
    x_in = nc.declare_dram_parameter("x", [H, T, C], f32, isOutput=False)
    y_out = nc.declare_dram_parameter("y", [H, T, C], bf16, isOutput=True)

    n = len(CHUNKS)
    TT_MAX = TC_MAX // P
    FREE_MAX = H * TT_MAX * C

    def tt_of(ci):
        return CHUNKS[ci][1] // P

    def m1b_of(ci):
        # 0 = all M1 slices on DVE. Offloading the last slice to ACT was
        # tested twice (two schedules) and regressed ~9us both times: ACT
        # touching xt/qt alongside DVE slows the shared-tensor ops ~20%.
        return 0

    # --- absolute dve_sem index per op, computed by simulating the
    # emission order of the vector block below -------------------------
    idx_reduce = [0] * n
    idx_recip = [0] * n     # recip (+ the inv2b scalar when m1b) done
    idx_m1_last = [0] * n
    idx_and = [0] * n
    _c = 0

    def _nxt():
        nonlocal _c
        _c += 1
        return _c

    def _sim_recip(ci):
        idx_recip[ci] = _nxt()          # reciprocal
        if m1b_of(ci):
            idx_recip[ci] = _nxt()      # inv2b = inv * sqrt2 ([P,1])

    idx_reduce[0] = _nxt()
    _sim_recip(0)
    for ci in range(n):
        # iter ci: reduce(ci+1), M1(ci) s0, recip+inv2b(ci+1), M1 rest, AND
        if ci + 1 < n:
            idx_reduce[ci + 1] = _nxt()
        for s in range(tt_of(ci) - m1b_of(ci)):
            idx_m1_last[ci] = _nxt()
            if s == 0 and ci + 1 < n:
                _sim_recip(ci + 1)
        idx_and[ci] = _nxt()

    ACT_END, ACTM1_END = [], []
    _a = _b = 0
    for ci in range(n):
        _a += tt_of(ci)
        _b += m1b_of(ci)
        ACT_END.append(_a)
        ACTM1_END.append(_b)

    def src_ap(ci):
        t0, tc = CHUNKS[ci]
        return x_in[:, t0 : t0 + tc, :].rearrange("h (p q) c -> p h (q c)", p=P)

    def dst_ap(ci):
        t0, tc = CHUNKS[ci]
        return y_out[:, t0 : t0 + tc, :].rearrange("h (p q) c -> p h (q c)", p=P)

    with ExitStack() as ctx:
        xt = [
            ctx.enter_context(nc.sbuf_tensor(f"xt{j}", [P, FREE_MAX], f32))
            for j in range(NBUF)
        ]
        wt = [
            ctx.enter_context(nc.sbuf_tensor(f"wt{j}", [P, FREE_MAX], bf16))
            for j in range(NBUF)
        ]
        qt = [
            ctx.enter_context(nc.sbuf_tensor(f"qt{k}", [P, FREE_MAX], f32))
            for k in range(ROLL)
        ]
        qt2 = [
            ctx.enter_context(nc.sbuf_tensor(f"qt2_{k}", [P, FREE_MAX], f32))
            for k in range(ROLL)
        ]
        delta = [
            ctx.enter_context(nc.sbuf_tensor(f"delta{j}", [P, TT_MAX], f32))
            for j in range(NBUF)
        ]
        # inv[:, :tt] = 1/delta; inv[:, TT_MAX] = inv2b = sqrt2/delta for
        # the ACT M1B slice (ACT's activation has only one scale slot)
        inv = [
            ctx.enter_context(nc.sbuf_tensor(f"inv{j}", [P, TT_MAX + 1], f32))
            for j in range(NBUF)
        ]
        warm = ctx.enter_context(nc.sbuf_tensor("warm", [P, 1], f32))

        load_sem = [
            ctx.enter_context(nc.semaphore(f"load_sem{j}")) for j in range(NBUF)
        ]
        store_sem = [
            ctx.enter_context(nc.semaphore(f"store_sem{j}")) for j in range(NBUF)
        ]
        dve_sem = ctx.enter_context(nc.semaphore("dve_sem"))
        act_m1_sem = ctx.enter_context(nc.semaphore("act_m1_sem"))
        act_sem = ctx.enter_context(nc.semaphore("act_sem"))

        block = ctx.enter_context(nc.Block())

        def views(buf, ci):
            return buf[:, : H * tt_of(ci) * C].rearrange(
                "p (h q c) -> p h q c", h=H, c=C
            )

        @block.sync
        def _(sync):
            for ci in range(n):
                j = ci % NBUF
                if ci >= NBUF:
                    # xt slot readers: reduce + M1 on DVE, M1B on ACT
                    sync.wait_ge(dve_sem, idx_and[ci - NBUF])
                    sync.wait_ge(act_m1_sem, ACTM1_END[ci - NBUF])
                sync.dma_start(
                    out=xt[j][:, : H * tt_of(ci) * C], in_=src_ap(ci)
                ).then_inc(load_sem[j], 16)

        def emit_reduce(vector, ci):
            j = ci % NBUF
            tt = tt_of(ci)
            vector.wait_ge(load_sem[j], 16 * (ci // NBUF + 1))
            if ci >= NBUF:
                # delta slot WAR: M2(ci-NBUF) read it (as scale)
                vector.wait_ge(act_sem, ACT_END[ci - NBUF])
            # delta = max over (h, c): one XY reduce on the [p, q, h, c]
            # transposed view
            vector.reduce_max(
                out=delta[j][:, :tt],
                in_=views(xt[j], ci).transpose([0, 2, 1, 3]),
                axis=mybir.AxisListType.XY,
            ).then_inc(dve_sem, 1)

        def emit_recip(vector, ci):
            j = ci % NBUF
            tt = tt_of(ci)
            vector.wait_ge(dve_sem, idx_reduce[ci])
            vector.reciprocal(inv[j][:, :tt], delta[j][:, :tt]).then_inc(
                dve_sem, 1
            )
            if m1b_of(ci):
                # per-token scalar for ACT's M1B slice ([P,1], ~65 cycles)
                s = tt - 1
                vector.wait_ge(dve_sem, idx_recip[ci] - 1)
                vector.tensor_scalar_mul(
                    inv[j][:, TT_MAX : TT_MAX + 1],
                    inv[j][:, s : s + 1],
                    SQRT2,
                ).then_inc(dve_sem, 1)

        @block.vector
        def _(vector):
            emit_reduce(vector, 0)
            emit_recip(vector, 0)
            for ci in range(n):
                j = ci % NBUF
                tt = tt_of(ci)
                xt4 = views(xt[j], ci)
                qt4 = views(qt[ci % ROLL], ci)

                if ci + 1 < n:
                    emit_reduce(vector, ci + 1)
                if ci >= ROLL:
                    # rolling qt/qt2 + delta WAR: M2(ci-ROLL) must have
                    # read them
                    vector.wait_ge(act_sem, ACT_END[ci - ROLL])
                vector.wait_ge(dve_sem, idx_recip[ci])
                for s in range(tt - m1b_of(ci)):
                    # M1: q = (x * inv) * sqrt2 (two-op tensor_scalar, 2x_2P)
                    vector.tensor_scalar(
                        out=qt4[:, :, s, :],
                        in0=xt4[:, :, s, :],
                        scalar1=inv[j][:, s : s + 1],
                        scalar2=SQRT2,
                        op0=OP.mult,
                        op1=OP.mult,
                    ).then_inc(dve_sem, 1)
                    if s == 0 and ci + 1 < n:
                        emit_recip(vector, ci + 1)
                # AND: p2 = bits(q) & EXP_MASK over the whole chunk (2x_2P);
                # the last M1 slice comes from ACT
                vector.wait_ge(dve_sem, idx_m1_last[ci])
                if m1b_of(ci):
                    vector.wait_ge(act_m1_sem, ACTM1_END[ci])
                vector.tensor_scalar(
                    out=qt2[ci % ROLL][:, : H * tt * C].bitcast(i32),
                    in0=qt[ci % ROLL][:, : H * tt * C].bitcast(i32),
                    scalar1=EXP_MASK,
                    scalar2=None,
                    op0=OP.bitwise_and,
                ).then_inc(dve_sem, 1)

        @block.scalar
        def _(scalar):
            # warm the ACT function table before the pipeline needs it
            scalar.activation(warm[:], warm[:], AF.Copy, scale=1.0)
            for ci in range(n):
                j = ci % NBUF
                tt = tt_of(ci)
                xt4 = views(xt[j], ci)
                qt4 = views(qt[ci % ROLL], ci)
                qt24 = views(qt2[ci % ROLL], ci)
                wt4 = views(wt[j], ci)

                if m1b_of(ci):
                    # M1B: last token-slice of q = x * (sqrt2/delta)
                    s = tt - 1
                    scalar.wait_ge(dve_sem, idx_recip[ci])
                    scalar.activation(
                        out=qt4[:, :, s, :],
                        in_=xt4[:, :, s, :],
                        func=AF.Copy,
                        scale=inv[j][:, TT_MAX : TT_MAX + 1],
                    ).then_inc(act_m1_sem, 1)
                # M2: out = p2 * delta with bf16 cast, then store
                scalar.wait_ge(dve_sem, idx_and[ci])
                if ci >= NBUF:
                    scalar.wait_ge(store_sem[j], 16 * (ci // NBUF))
                for s in range(tt):
                    scalar.activation(
                        out=wt4[:, :, s, :],
                        in_=qt24[:, :, s, :],
                        func=AF.Copy,
                        scale=delta[j][:, s : s + 1],
                    ).then_inc(act_sem, 1)
                # self-fence: M2 writes must land in SBUF before the DMA
                scalar.wait_ge(act_sem, ACT_END[ci])
                scalar.dma_start(
                    out=dst_ap(ci), in_=wt[j][:, : H * tt * C]
                ).then_inc(store_sem[j], 16)

    _nc_cache["nc"] = nc
    return nc


def kernel(x: np.ndarray) -> np.ndarray:
    assert x.shape == (B, H, T, C) and x.dtype == np.float32
    nc = _build_nc()
    in_maps = [{"x": np.ascontiguousarray(x[i])} for i in range(N_CORES)]
    res = run_bass_kernel_spmd(nc, in_maps, list(range(N_CORES)))
    out = np.stack(
        [np.asarray(res.results[i]["y"]).astype(np.float32) for i in range(N_CORES)],
        axis=0,
    )
    return out
